# revision 17
# baseline (speedup 1.0000x reference)
"""Trainium2 Bass kernel for DiffAttention (nn_DiffAttention_49847390437777).

Contract: kernel(**full_inputs) -> full output [2, 2048, 8, 256] fp32.

Sharding (8 cores): core c handles batch b = c//4 and global query-head pairs
{2*(c%4), 2*(c%4)+1} (i.e. heads 4*(c%4)..4*(c%4)+3).  Diff-attention couples
only adjacent head pairs, which stay co-located.  lambda scalars are computed
on host and shipped as a tiny replicated tensor; subln_weight is applied on
host after the gather (it multiplies AFTER the RMS norm, so this is exact).

Device algorithm per core (4 heads = 2 pairs, seq 2048, head_dim 128):
  - scores computed transposed: S^T[k, q] = kT_blk.T @ qT_blk (contraction on
    d=128 partitions), causal blocks only.
  - softmax without max-subtraction (scores are O(1); exp is safe) so the
    row-sum reduction is along PSUM partitions -> fused into the PV matmul via
    a ones-column appended to V:  O = P^T.T @ [v1|v2|1]  gives the unnormalized
    attention and the rowsum in one accumulation group.
  - P^T is exp'd on ACT (PSUM->SBUF) in bf16; the causal diagonal 128x128
    block is masked with a precomputed triangular tile on DVE.
  - the first q-superblock (rows 0..511) runs in float32r instead of bf16:
    there attn1 ~= lambda*attn2 (strong cancellation) and bf16 rounding noise
    would be amplified by the RMS norm.
  - epilogue: r = 1/rowsum (DVE reciprocal), A1 = O1*r1 kept fp32,
    diff = A1 - (lambda*r2)*O2 computed in fp32, stored bf16;
    ssum = sum(diff^2) via tensor_mul + reduce_sum (the fused
    tensor_tensor_reduce custom-DVE op crashes the exec unit on this path);
    rms = exp(-0.5*ln(ssum/256 + eps) + ln(1-lambda_init)) on ACT
    (exp+ln share one ACT table set; ACT Rsqrt is banned for accuracy);
    out = diff * rms -> DMA.
"""

import math
import os

import numpy as np
import ml_dtypes

HEAD_DIM = 128
N_HEADS = 16
LAYER_IDX = 12
LAMBDA_INIT = 0.8 - 0.6 * math.exp(-0.3 * (LAYER_IDX - 1))
EPS = 1e-5
SCALE = 1.0 / math.sqrt(HEAD_DIM)
S_FOLD = 1.0 - LAMBDA_INIT

B = 2
S = 2048
NB = S // 128   # 16 key blocks of 128
QB = S // 512   # 4 query superblocks of 512
N_CORES = 8

bf16 = ml_dtypes.bfloat16

_CACHE = {}
last_results = None  # BassKernelResults of the most recent run (for test.py)


def build_nc(hiprec=True):
    """Build + compile the per-core Bass program (same program on all cores)."""
    import concourse.bass as bass
    import concourse.mybir as mybir
    import concourse.bacc as bacc
    import concourse.tile as tile
    from concourse.masks import make_upper_triangular
    from contextlib import ExitStack

    f32 = mybir.dt.float32
    f32r = mybir.dt.float32r
    b16 = mybir.dt.bfloat16
    AF = mybir.ActivationFunctionType
    ALU = mybir.AluOpType

    nc = bacc.Bacc("TRN2", target_bir_lowering=False, debug=False)

    # consolidated inputs: one fat DMA each (a dma_start fans out across all
    # 16 SDMA engines; many small DMAs pay ~2us completion latency each)
    hp32 = nc.dram_tensor("hp32", [2, 128, 2, 1024], f32, kind="ExternalInput")
    qkb = nc.dram_tensor("qkb", [2, 128, 2, 4096], b16, kind="ExternalInput")
    vxb = nc.dram_tensor("vxb", [2, 128, NB, 257], b16, kind="ExternalInput")
    vx32 = nc.dram_tensor("vx32", [2, 128, 4, 258], f32, kind="ExternalInput")
    lam = nc.dram_tensor("lam", [128, 1], f32, kind="ExternalInput")
    o = nc.dram_tensor("o", [2, NB, 128, 256], f32, kind="ExternalOutput")

    with tile.TileContext(nc) as tc:
        with ExitStack() as ctx:
            ec = ctx.enter_context
            const = ec(tc.tile_pool(name="const", bufs=1))
            qkpool = ec(tc.tile_pool(name="qkpool", bufs=2))
            hppool = ec(tc.tile_pool(name="hppool", bufs=2))
            q3rpool = ec(tc.tile_pool(name="q3rpool", bufs=2))
            k3rpool = ec(tc.tile_pool(name="k3rpool", bufs=2))
            vpool = ec(tc.tile_pool(name="vpool", bufs=2))
            v32pool = ec(tc.tile_pool(name="v32pool", bufs=2))
            ppool = ec(tc.tile_pool(name="ppool", bufs=3))
            p32pool = ec(tc.tile_pool(name="p32pool", bufs=2))
            apool = ec(tc.tile_pool(name="apool", bufs=2))
            dpool = ec(tc.tile_pool(name="dpool", bufs=2))
            stat = ec(tc.tile_pool(name="stat", bufs=3))
            tmp = ec(tc.tile_pool(name="tmp", bufs=3))
            opool = ec(tc.tile_pool(name="opool", bufs=4))
            spsum = ec(tc.tile_pool(name="spsum", bufs=2, space="PSUM"))
            opsum = ec(tc.tile_pool(name="opsum", bufs=4, space="PSUM"))

            tri16 = const.tile([128, 128], b16)
            make_upper_triangular(nc, tri16[:], val=1.0, diag=True)
            tri32 = const.tile([128, 128], f32)
            make_upper_triangular(nc, tri32[:], val=1.0, diag=True)
            lamt = const.tile([128, 1], f32)
            nc.gpsimd.dma_start(lamt[:], lam[:])
            eps_t = const.tile([128, 1], f32)
            nc.gpsimd.memset(eps_t[:], EPS)
            lsf_t = const.tile([128, 1], f32)
            nc.gpsimd.memset(lsf_t[:], math.log(S_FOLD))

            for pair in range(2):
                # critical first: fp32 inputs for the qb0 high-precision path
                hp_t = hppool.tile([128, 2, 1024], f32, tag="hp", name="hp_t")
                nc.sync.dma_start(hp_t[:], hp32[pair])
                qt3, kt3 = {}, {}
                for par in range(2):
                    qt3[par] = q3rpool.tile([128, 512], f32r, tag="qt3r", name="qt3r")
                    nc.vector.tensor_copy(qt3[par][:], hp_t[:, par, 0:512])
                    kt3[par] = k3rpool.tile([128, 512], f32r, tag="kt3r", name="kt3r")
                    nc.vector.tensor_copy(kt3[par][:], hp_t[:, par, 512:1024])

                vx_3s = v32pool.tile([128, 4, 258], f32, tag="vx32s", name="vx_3s")
                nc.gpsimd.dma_start(vx_3s[:], vx32[pair])
                vx_3 = v32pool.tile([128, 4, 258], f32r, tag="vx32", name="vx_3")
                nc.vector.tensor_copy(vx_3[:], vx_3s[:])
                vx_b = vpool.tile([128, NB, 257], b16, tag="vx")
                nc.gpsimd.dma_start(vx_b[:], vxb[pair])

                qk_t = qkpool.tile([128, 2, 4096], b16, tag="qk", name="qk_t")
                nc.sync.dma_start(qk_t[:], qkb[pair])
                qt = {par: qk_t[:, par, 0:2048] for par in range(2)}
                kt = {par: qk_t[:, par, 2048:4096] for par in range(2)}

                # software pipeline: scores/exp/mask for qb, then PV for qb-1
                prev = None  # (qb, {par: pt tile}, is32)
                for qb in range(QB + 1):
                    if qb < QB:
                        q0 = qb * 512
                        nkb = 4 * qb + 4
                        cur = {}
                        for par in range(2):
                            if qb == 0 and hiprec:
                                p3 = p32pool.tile([128, 4, 512], f32r, tag="pt32", name="pt32")
                                cur[par] = p3
                                for g in range(2):
                                    sp = spsum.tile([128, 2, 512], f32, tag="sp")
                                    for t in range(2):
                                        kb = 2 * g + t
                                        qoff = kb * 128
                                        nc.tensor.matmul(
                                            sp[:, t, qoff:512],
                                            kt3[par][:, kb * 128:(kb + 1) * 128],
                                            qt3[par][:, qoff:512],
                                            start=True, stop=True,
                                        )
                                    for t in range(2):
                                        kb = 2 * g + t
                                        qoff = kb * 128
                                        nc.scalar.activation(
                                            p3[:, kb, qoff:512], sp[:, t, qoff:512],
                                            AF.Exp, scale=SCALE,
                                        )
                                        nc.vector.tensor_mul(
                                            p3[:, kb, qoff:qoff + 128],
                                            p3[:, kb, qoff:qoff + 128], tri32[:],
                                        )
                            else:
                                p1 = ppool.tile([128, NB, 512], b16, tag="pt", name="pt")
                                cur[par] = p1
                                for g in range(nkb // 2):
                                    sp = spsum.tile([128, 2, 512], f32, tag="sp")
                                    for t in range(2):
                                        kb = 2 * g + t
                                        qoff = 0 if kb < 4 * qb else (kb - 4 * qb) * 128
                                        nc.tensor.matmul(
                                            sp[:, t, qoff:512],
                                            kt[par][:, kb * 128:(kb + 1) * 128],
                                            qt[par][:, q0 + qoff:q0 + 512],
                                            start=True, stop=True,
                                        )
                                    if 2 * g + 1 < 4 * qb:
                                        nc.scalar.activation(
                                            p1[:, 2 * g:2 * g + 2, :], sp[:, :, :],
                                            AF.Exp, scale=SCALE,
                                        )
                                    else:
                                        for t in range(2):
                                            kb = 2 * g + t
                                            qoff = 0 if kb < 4 * qb else (kb - 4 * qb) * 128
                                            nc.scalar.activation(
                                                p1[:, kb, qoff:512], sp[:, t, qoff:512],
                                                AF.Exp, scale=SCALE,
                                            )
                                    for t in range(2):
                                        kb = 2 * g + t
                                        if kb >= 4 * qb:
                                            qoff = (kb - 4 * qb) * 128
                                            nc.vector.tensor_mul(
                                                p1[:, kb, qoff:qoff + 128],
                                                p1[:, kb, qoff:qoff + 128], tri16[:],
                                            )
                        nxt = (qb, cur, qb == 0 and hiprec)
                    else:
                        nxt = None
                    if prev is not None:
                        pqb, ppt, is32 = prev
                        A1q = apool.tile([128, 4, 256], f32, tag="A1", name="A1q")
                        diffq = dpool.tile([128, 4, 256], b16, tag="diff", name="diffq")
                        ssq = stat.tile([128, 4], f32, tag="ssq", name="ssq")
                        for par in range(2):
                            for j in range(4):
                                jabs = 4 * pqb + j
                                op_t = opsum.tile([128, 258], f32, tag="op")
                                for kb in range(jabs + 1):
                                    if is32:
                                        lhsT = ppt[par][:, kb, j * 128:(j + 1) * 128]
                                        rhs = vx_3[:, kb, :]
                                    else:
                                        lhsT = ppt[par][:, kb, j * 128:(j + 1) * 128]
                                        rhs = vx_b[:, kb, :]
                                    nc.tensor.matmul(
                                        op_t[:, 0:rhs.shape[-1]], lhsT, rhs,
                                        start=(kb == 0), stop=(kb == jabs),
                                    )
                                rc = tmp.tile([128, 1], f32, tag="rc")
                                nc.vector.reciprocal(rc[:], op_t[:, 256:257])
                                if par == 0:
                                    nc.vector.tensor_scalar_mul(
                                        A1q[:, j, :], op_t[:, 0:256], rc[:])
                                else:
                                    rcl = tmp.tile([128, 1], f32, tag="rcl")
                                    nc.vector.tensor_mul(rcl[:], rc[:], lamt[:])
                                    t2 = tmp.tile([128, 256], f32, tag="t2")
                                    nc.vector.tensor_scalar_mul(
                                        t2[:], op_t[:, 0:256], rcl[:])
                                    nc.vector.tensor_sub(
                                        diffq[:, j, :], A1q[:, j, :], t2[:])
                                    sq = tmp.tile([128, 256], b16, tag="sq")
                                    nc.vector.tensor_mul(
                                        sq[:], diffq[:, j, :], diffq[:, j, :])
                                    nc.vector.reduce_sum(
                                        ssq[:, j:j + 1], sq[:],
                                        axis=mybir.AxisListType.X)
                        # per-qb finalization: rms + scale + store
                        lnm = stat.tile([128, 4], f32, tag="lnm", name="lnm")
                        nc.scalar.activation(lnm[:], ssq[:], AF.Ln,
                                             scale=1.0 / 256.0, bias=eps_t[:])
                        rmst = stat.tile([128, 4], f32, tag="rms", name="rmst")
                        nc.scalar.activation(rmst[:], lnm[:], AF.Exp,
                                             scale=-0.5, bias=lsf_t[:])
                        for j in range(4):
                            ot = opool.tile([128, 256], f32, tag="ot")
                            nc.vector.tensor_scalar_mul(
                                ot[:], diffq[:, j, :], rmst[:, j:j + 1])
                            nc.gpsimd.dma_start(o[pair, 4 * pqb + j], ot[:])
                    prev = nxt

    # Pin Exp+Ln to the one table set containing both
    # (natural_log_exp_and_others) — the greedy per-function chooser otherwise
    # thrashes between exp_and_others and the ln set (~1.3us per reload, and it
    # serializes the pipeline around each switch).
    _orig_gat = bacc.get_activation_tables

    def _gat(arch):
        tabs = _orig_gat(arch)
        for name, fns in tabs.items():
            if name != "natural_log_exp_and_others":
                fns.discard(AF.Exp)
                fns.discard(AF.Ln)
        return tabs

    bacc.get_activation_tables = _gat
    try:
        nc.compile()
    finally:
        bacc.get_activation_tables = _orig_gat
    return nc


def _prep_core_inputs(q, k, v, lam_full):
    """Host-side shard + layout prep. Returns list of 8 per-core input dicts."""
    in_maps = []
    for c in range(N_CORES):
        b = c // 4
        h0 = 4 * (c % 4)
        # [s, 4, d] -> [4, d, s]
        qs = np.ascontiguousarray(q[b, :, h0:h0 + 4, :].transpose(1, 2, 0))
        ks = np.ascontiguousarray(k[b, :, h0:h0 + 4, :].transpose(1, 2, 0))
        # qkb: [pair, p, par, qT 2048 | kT 2048] bf16
        qkb_ = np.empty((2, 128, 2, 4096), bf16)
        for pair in range(2):
            for par in range(2):
                h = 2 * pair + par
                qkb_[pair, :, par, 0:2048] = qs[h].astype(bf16)
                qkb_[pair, :, par, 2048:4096] = ks[h].astype(bf16)
        # hp32: [pair, p, par, q512 | k512] fp32
        hp32_ = np.empty((2, 128, 2, 1024), np.float32)
        for pair in range(2):
            for par in range(2):
                h = 2 * pair + par
                hp32_[pair, :, par, 0:512] = qs[h][:, :512]
                hp32_[pair, :, par, 512:1024] = ks[h][:, :512]
        vx = np.empty((2, S, 257), np.float32)
        for pair in range(2):
            vx[pair, :, :128] = v[b, :, h0 + 2 * pair, :]
            vx[pair, :, 128:256] = v[b, :, h0 + 2 * pair + 1, :]
            vx[pair, :, 256] = 1.0
        # [2, s, 257] -> partition-major [2, 128, nb, 257]
        vxp = vx.reshape(2, NB, 128, 257).transpose(0, 2, 1, 3)
        vxb_ = np.ascontiguousarray(vxp).astype(bf16)
        vx32_ = np.zeros((2, 128, 4, 258), np.float32)
        vx32_[:, :, :, :257] = vxp[:, :, :4, :]
        lam_t = np.full((128, 1), lam_full, np.float32)
        in_maps.append({
            "qkb": qkb_, "hp32": hp32_,
            "vxb": vxb_, "vx32": vx32_, "lam": lam_t,
        })
    return in_maps


def kernel(q, k, v, lambda_q1, lambda_k1, lambda_q2, lambda_k2,
           subln_weight, attention_mask):
    global last_results
    from concourse.bass_utils import run_bass_kernel_spmd

    q = np.ascontiguousarray(np.asarray(q, np.float32))
    k = np.ascontiguousarray(np.asarray(k, np.float32))
    v = np.ascontiguousarray(np.asarray(v, np.float32))
    lam1 = np.exp(np.sum(np.asarray(lambda_q1, np.float32)
                         * np.asarray(lambda_k1, np.float32), dtype=np.float32))
    lam2 = np.exp(np.sum(np.asarray(lambda_q2, np.float32)
                         * np.asarray(lambda_k2, np.float32), dtype=np.float32))
    lam_full = np.float32(lam1 - lam2 + np.float32(LAMBDA_INIT))

    if "nc" not in _CACHE:
        _CACHE["nc"] = build_nc()
    nc = _CACHE["nc"]

    in_maps = _prep_core_inputs(q, k, v, lam_full)
    trace = bool(int(os.environ.get("KERNEL_TRACE", "0")))
    kw = {}
    if trace:
        kw = dict(trace=True, trace_cores=list(range(N_CORES)))
    res = run_bass_kernel_spmd(nc, in_maps, core_ids=list(range(N_CORES)), **kw)
    last_results = res

    out = np.empty((B, S, N_HEADS // 2, 256), np.float32)
    for c in range(N_CORES):
        b = c // 4
        gp = 2 * (c % 4)
        oc = res.results[c]["o"].reshape(2, S, 256)  # [pair, s, 256]
        out[b, :, gp, :] = oc[0]
        out[b, :, gp + 1, :] = oc[1]
    out *= np.asarray(subln_weight, np.float32)[None, None, None, :]
    return out


# revision 19
# speedup vs baseline: 1.1160x; 1.1160x over previous
"""Trainium2 Bass kernel for DiffAttention (nn_DiffAttention_49847390437777).

Contract: kernel(**full_inputs) -> full output [2, 2048, 8, 256] fp32.

Sharding (8 cores): core c handles batch b = c//4 and global query-head pairs
{2*(c%4), 2*(c%4)+1} (i.e. heads 4*(c%4)..4*(c%4)+3).  Diff-attention couples
only adjacent head pairs, which stay co-located.  lambda scalars are computed
on host and shipped as a tiny replicated tensor; subln_weight is applied on
host after the gather (it multiplies AFTER the RMS norm, so this is exact).

Device algorithm per core (4 heads = 2 pairs, seq 2048, head_dim 128):
  - scores computed transposed: S^T[k, q] = kT_blk.T @ qT_blk (contraction on
    d=128 partitions), causal blocks only.
  - softmax without max-subtraction (scores are O(1); exp is safe) so the
    row-sum reduction is along PSUM partitions -> fused into the PV matmul via
    a ones-column appended to V:  O = P^T.T @ [v1|v2|1]  gives the unnormalized
    attention and the rowsum in one accumulation group.
  - P^T is exp'd on ACT (PSUM->SBUF) in bf16; the causal diagonal 128x128
    block is masked with a precomputed triangular tile on DVE.
  - the first q-superblock (rows 0..511) runs in float32r instead of bf16:
    there attn1 ~= lambda*attn2 (strong cancellation) and bf16 rounding noise
    would be amplified by the RMS norm.
  - epilogue: r = 1/rowsum (DVE reciprocal), A1 = O1*r1 kept fp32,
    diff = A1 - (lambda*r2)*O2 computed in fp32, stored bf16;
    ssum = sum(diff^2) via tensor_mul + reduce_sum (the fused
    tensor_tensor_reduce custom-DVE op crashes the exec unit on this path);
    rms = exp(-0.5*ln(ssum/256 + eps) + ln(1-lambda_init)) on ACT
    (exp+ln share one ACT table set; ACT Rsqrt is banned for accuracy);
    out = diff * rms -> DMA.
"""

import math
import os

import numpy as np
import ml_dtypes

HEAD_DIM = 128
N_HEADS = 16
LAYER_IDX = 12
LAMBDA_INIT = 0.8 - 0.6 * math.exp(-0.3 * (LAYER_IDX - 1))
EPS = 1e-5
SCALE = 1.0 / math.sqrt(HEAD_DIM)
S_FOLD = 1.0 - LAMBDA_INIT

B = 2
S = 2048
NB = S // 128   # 16 key blocks of 128
QB = S // 512   # 4 query superblocks of 512
N_CORES = 8

bf16 = ml_dtypes.bfloat16

_CACHE = {}
last_results = None  # BassKernelResults of the most recent run (for test.py)


def build_nc(hiprec=True):
    """Build + compile the per-core Bass program (same program on all cores)."""
    import concourse.bass as bass
    import concourse.mybir as mybir
    import concourse.bacc as bacc
    import concourse.tile as tile
    from concourse.masks import make_upper_triangular
    from contextlib import ExitStack

    f32 = mybir.dt.float32
    f32r = mybir.dt.float32r
    b16 = mybir.dt.bfloat16
    AF = mybir.ActivationFunctionType
    ALU = mybir.AluOpType

    nc = bacc.Bacc("TRN2", target_bir_lowering=False, debug=False)

    # consolidated inputs: one fat DMA each (a dma_start fans out across all
    # 16 SDMA engines; many small DMAs pay ~2us completion latency each)
    hp32 = nc.dram_tensor("hp32", [2, 128, 2, 1024], f32, kind="ExternalInput")
    qkb = nc.dram_tensor("qkb", [2, 2, 128, 4096], b16, kind="ExternalInput")
    vxb = nc.dram_tensor("vxb", [2, 128, NB, 257], b16, kind="ExternalInput")
    vx32 = nc.dram_tensor("vx32", [2, 128, 4, 260], f32, kind="ExternalInput")
    o = nc.dram_tensor("o", [2, NB, 128, 256], f32, kind="ExternalOutput")

    with tile.TileContext(nc) as tc:
        with ExitStack() as ctx:
            ec = ctx.enter_context
            const = ec(tc.tile_pool(name="const", bufs=1))
            qkpool = ec(tc.tile_pool(name="qkpool", bufs=2))
            hppool = ec(tc.tile_pool(name="hppool", bufs=2))
            q3rpool = ec(tc.tile_pool(name="q3rpool", bufs=2))
            k3rpool = ec(tc.tile_pool(name="k3rpool", bufs=2))
            vpool = ec(tc.tile_pool(name="vpool", bufs=2))
            v32pool = ec(tc.tile_pool(name="v32pool", bufs=2))
            ppool = ec(tc.tile_pool(name="ppool", bufs=3))
            p32pool = ec(tc.tile_pool(name="p32pool", bufs=2))
            apool = ec(tc.tile_pool(name="apool", bufs=2))
            dpool = ec(tc.tile_pool(name="dpool", bufs=2))
            stat = ec(tc.tile_pool(name="stat", bufs=3))
            tmp = ec(tc.tile_pool(name="tmp", bufs=3))
            opool = ec(tc.tile_pool(name="opool", bufs=4))
            spsum = ec(tc.tile_pool(name="spsum", bufs=2, space="PSUM"))
            opsum = ec(tc.tile_pool(name="opsum", bufs=4, space="PSUM"))

            tri16 = const.tile([128, 128], b16)
            make_upper_triangular(nc, tri16[:], val=1.0, diag=True)
            tri32 = const.tile([128, 128], f32)
            make_upper_triangular(nc, tri32[:], val=1.0, diag=True)
            eps_t = const.tile([128, 1], f32)
            nc.gpsimd.memset(eps_t[:], EPS)
            lsf_t = const.tile([128, 1], f32)
            nc.gpsimd.memset(lsf_t[:], math.log(S_FOLD))

            for pair in range(2):
                # critical first: fp32 inputs for the qb0 high-precision path
                hp_t = hppool.tile([128, 2, 1024], f32, tag="hp", name="hp_t")
                nc.sync.dma_start(hp_t[:], hp32[pair])
                qt3, kt3 = {}, {}
                for par in range(2):
                    qt3[par] = q3rpool.tile([128, 512], f32r, tag="qt3r", name="qt3r")
                    nc.vector.tensor_copy(qt3[par][:], hp_t[:, par, 0:512])
                    kt3[par] = k3rpool.tile([128, 512], f32r, tag="kt3r", name="kt3r")
                    nc.vector.tensor_copy(kt3[par][:], hp_t[:, par, 512:1024])

                vx_3s = v32pool.tile([128, 4, 260], f32, tag="vx32s", name="vx_3s")
                nc.sync.dma_start(vx_3s[:], vx32[pair])
                vx_3 = v32pool.tile([128, 4, 260], f32r, tag="vx32", name="vx_3")
                nc.vector.tensor_copy(vx_3[:], vx_3s[:])
                lamt = vx_3s[:, 0, 258:259]
                vx_b = vpool.tile([128, NB, 257], b16, tag="vx")
                nc.gpsimd.dma_start(vx_b[:], vxb[pair])

                qk_t = qkpool.tile([128, 2, 4096], b16, tag="qk", name="qk_t")
                for par in range(2):
                    nc.sync.dma_start(qk_t[:, par, :], qkb[pair, par])
                qt = {par: qk_t[:, par, 0:2048] for par in range(2)}
                kt = {par: qk_t[:, par, 2048:4096] for par in range(2)}

                # software pipeline: scores/exp/mask for qb, then PV for qb-1
                prev = None  # (qb, {par: pt tile}, is32)
                for qb in range(QB + 1):
                    if qb < QB:
                        q0 = qb * 512
                        nkb = 4 * qb + 4
                        cur = {}
                        for par in range(2):
                            if qb == 0 and hiprec:
                                p3 = p32pool.tile([128, 4, 512], f32r, tag="pt32", name="pt32")
                                cur[par] = p3
                                for g in range(2):
                                    sp = spsum.tile([128, 2, 512], f32, tag="sp")
                                    for t in range(2):
                                        kb = 2 * g + t
                                        qoff = kb * 128
                                        nc.tensor.matmul(
                                            sp[:, t, qoff:512],
                                            kt3[par][:, kb * 128:(kb + 1) * 128],
                                            qt3[par][:, qoff:512],
                                            start=True, stop=True,
                                        )
                                    for t in range(2):
                                        kb = 2 * g + t
                                        qoff = kb * 128
                                        nc.scalar.activation(
                                            p3[:, kb, qoff:512], sp[:, t, qoff:512],
                                            AF.Exp, scale=SCALE,
                                        )
                                        nc.vector.tensor_mul(
                                            p3[:, kb, qoff:qoff + 128],
                                            p3[:, kb, qoff:qoff + 128], tri32[:],
                                        )
                            else:
                                p1 = ppool.tile([128, NB, 512], b16, tag="pt", name="pt")
                                cur[par] = p1
                                for g in range(nkb // 2):
                                    sp = spsum.tile([128, 2, 512], f32, tag="sp")
                                    for t in range(2):
                                        kb = 2 * g + t
                                        qoff = 0 if kb < 4 * qb else (kb - 4 * qb) * 128
                                        nc.tensor.matmul(
                                            sp[:, t, qoff:512],
                                            kt[par][:, kb * 128:(kb + 1) * 128],
                                            qt[par][:, q0 + qoff:q0 + 512],
                                            start=True, stop=True,
                                        )
                                    if 2 * g + 1 < 4 * qb:
                                        nc.scalar.activation(
                                            p1[:, 2 * g:2 * g + 2, :], sp[:, :, :],
                                            AF.Exp, scale=SCALE,
                                        )
                                    else:
                                        for t in range(2):
                                            kb = 2 * g + t
                                            qoff = 0 if kb < 4 * qb else (kb - 4 * qb) * 128
                                            nc.scalar.activation(
                                                p1[:, kb, qoff:512], sp[:, t, qoff:512],
                                                AF.Exp, scale=SCALE,
                                            )
                                    for t in range(2):
                                        kb = 2 * g + t
                                        if kb >= 4 * qb:
                                            qoff = (kb - 4 * qb) * 128
                                            nc.vector.tensor_mul(
                                                p1[:, kb, qoff:qoff + 128],
                                                p1[:, kb, qoff:qoff + 128], tri16[:],
                                            )
                        nxt = (qb, cur, qb == 0 and hiprec)
                    else:
                        nxt = None
                    if prev is not None:
                        pqb, ppt, is32 = prev
                        A1q = apool.tile([128, 4, 256], f32, tag="A1", name="A1q")
                        diffq = dpool.tile([128, 4, 256], b16, tag="diff", name="diffq")
                        ssq = stat.tile([128, 4], f32, tag="ssq", name="ssq")
                        for par in range(2):
                            for j in range(4):
                                jabs = 4 * pqb + j
                                op_t = opsum.tile([128, 258], f32, tag="op")
                                for kb in range(jabs + 1):
                                    if is32:
                                        lhsT = ppt[par][:, kb, j * 128:(j + 1) * 128]
                                        rhs = vx_3[:, kb, 0:258]
                                    else:
                                        lhsT = ppt[par][:, kb, j * 128:(j + 1) * 128]
                                        rhs = vx_b[:, kb, :]
                                    nc.tensor.matmul(
                                        op_t[:, 0:rhs.shape[-1]], lhsT, rhs,
                                        start=(kb == 0), stop=(kb == jabs),
                                    )
                                rc = tmp.tile([128, 1], f32, tag="rc")
                                nc.vector.reciprocal(rc[:], op_t[:, 256:257])
                                if par == 0:
                                    nc.vector.tensor_scalar_mul(
                                        A1q[:, j, :], op_t[:, 0:256], rc[:])
                                else:
                                    rcl = tmp.tile([128, 1], f32, tag="rcl")
                                    nc.vector.tensor_mul(rcl[:], rc[:], lamt[:])
                                    t2 = tmp.tile([128, 256], f32, tag="t2")
                                    nc.vector.tensor_scalar_mul(
                                        t2[:], op_t[:, 0:256], rcl[:])
                                    nc.vector.tensor_sub(
                                        diffq[:, j, :], A1q[:, j, :], t2[:])
                                    sq = tmp.tile([128, 256], b16, tag="sq")
                                    nc.vector.tensor_mul(
                                        sq[:], diffq[:, j, :], diffq[:, j, :])
                                    nc.vector.reduce_sum(
                                        ssq[:, j:j + 1], sq[:],
                                        axis=mybir.AxisListType.X)
                        # per-qb finalization: rms + scale + store
                        lnm = stat.tile([128, 4], f32, tag="lnm", name="lnm")
                        nc.scalar.activation(lnm[:], ssq[:], AF.Ln,
                                             scale=1.0 / 256.0, bias=eps_t[:])
                        rmst = stat.tile([128, 4], f32, tag="rms", name="rmst")
                        nc.scalar.activation(rmst[:], lnm[:], AF.Exp,
                                             scale=-0.5, bias=lsf_t[:])
                        for j in range(4):
                            ot = opool.tile([128, 256], f32, tag="ot")
                            nc.vector.tensor_scalar_mul(
                                ot[:], diffq[:, j, :], rmst[:, j:j + 1])
                            nc.gpsimd.dma_start(o[pair, 4 * pqb + j], ot[:])
                    prev = nxt

    # Pin Exp+Ln to the one table set containing both
    # (natural_log_exp_and_others) — the greedy per-function chooser otherwise
    # thrashes between exp_and_others and the ln set (~1.3us per reload, and it
    # serializes the pipeline around each switch).
    _orig_gat = bacc.get_activation_tables

    def _gat(arch):
        tabs = _orig_gat(arch)
        for name, fns in tabs.items():
            if name != "natural_log_exp_and_others":
                fns.discard(AF.Exp)
                fns.discard(AF.Ln)
        return tabs

    bacc.get_activation_tables = _gat
    try:
        nc.compile()
    finally:
        bacc.get_activation_tables = _orig_gat
    return nc


def _prep_core_inputs(q, k, v, lam_full):
    """Host-side shard + layout prep. Returns list of 8 per-core input dicts."""
    in_maps = []
    for c in range(N_CORES):
        b = c // 4
        h0 = 4 * (c % 4)
        # [s, 4, d] -> [4, d, s]
        qs = np.ascontiguousarray(q[b, :, h0:h0 + 4, :].transpose(1, 2, 0))
        ks = np.ascontiguousarray(k[b, :, h0:h0 + 4, :].transpose(1, 2, 0))
        # qkb: [pair, par, p, qT 2048 | kT 2048] bf16
        qkb_ = np.empty((2, 2, 128, 4096), bf16)
        for pair in range(2):
            for par in range(2):
                h = 2 * pair + par
                qkb_[pair, par, :, 0:2048] = qs[h].astype(bf16)
                qkb_[pair, par, :, 2048:4096] = ks[h].astype(bf16)
        # hp32: [pair, p, par, q512 | k512] fp32
        hp32_ = np.empty((2, 128, 2, 1024), np.float32)
        for pair in range(2):
            for par in range(2):
                h = 2 * pair + par
                hp32_[pair, :, par, 0:512] = qs[h][:, :512]
                hp32_[pair, :, par, 512:1024] = ks[h][:, :512]
        vx = np.empty((2, S, 257), np.float32)
        for pair in range(2):
            vx[pair, :, :128] = v[b, :, h0 + 2 * pair, :]
            vx[pair, :, 128:256] = v[b, :, h0 + 2 * pair + 1, :]
            vx[pair, :, 256] = 1.0
        # [2, s, 257] -> partition-major [2, 128, nb, 257]
        vxp = vx.reshape(2, NB, 128, 257).transpose(0, 2, 1, 3)
        vxb_ = np.ascontiguousarray(vxp).astype(bf16)
        vx32_ = np.zeros((2, 128, 4, 260), np.float32)
        vx32_[:, :, :, :257] = vxp[:, :, :4, :]
        vx32_[:, :, :, 258] = lam_full
        in_maps.append({
            "qkb": qkb_, "hp32": hp32_,
            "vxb": vxb_, "vx32": vx32_,
        })
    return in_maps


def kernel(q, k, v, lambda_q1, lambda_k1, lambda_q2, lambda_k2,
           subln_weight, attention_mask):
    global last_results
    from concourse.bass_utils import run_bass_kernel_spmd

    q = np.ascontiguousarray(np.asarray(q, np.float32))
    k = np.ascontiguousarray(np.asarray(k, np.float32))
    v = np.ascontiguousarray(np.asarray(v, np.float32))
    lam1 = np.exp(np.sum(np.asarray(lambda_q1, np.float32)
                         * np.asarray(lambda_k1, np.float32), dtype=np.float32))
    lam2 = np.exp(np.sum(np.asarray(lambda_q2, np.float32)
                         * np.asarray(lambda_k2, np.float32), dtype=np.float32))
    lam_full = np.float32(lam1 - lam2 + np.float32(LAMBDA_INIT))

    if "nc" not in _CACHE:
        _CACHE["nc"] = build_nc()
    nc = _CACHE["nc"]

    in_maps = _prep_core_inputs(q, k, v, lam_full)
    trace = bool(int(os.environ.get("KERNEL_TRACE", "0")))
    kw = {}
    if trace:
        kw = dict(trace=True, trace_cores=list(range(N_CORES)))
    res = run_bass_kernel_spmd(nc, in_maps, core_ids=list(range(N_CORES)), **kw)
    last_results = res

    out = np.empty((B, S, N_HEADS // 2, 256), np.float32)
    for c in range(N_CORES):
        b = c // 4
        gp = 2 * (c % 4)
        oc = res.results[c]["o"].reshape(2, S, 256)  # [pair, s, 256]
        out[b, :, gp, :] = oc[0]
        out[b, :, gp + 1, :] = oc[1]
    out *= np.asarray(subln_weight, np.float32)[None, None, None, :]
    return out
